# revision 1
# baseline (speedup 1.0000x reference)
"""DilatedAttention Trainium2 kernel.

B=2, n=16 heads, T=8192, d=64. Three dilated passes (S,r) in
[(512,1),(1024,2),(2048,4)]; head h uses segments (h%r)+r*j; causal
softmax inside each segment; out = (p1+p2+p3)/3.

Sharding: 32 (b,h) pairs -> 8 cores x 4 pairs. A per-head block
permutation + duplication on the host makes the on-device program
head-independent (pure SPMD):
  slots 0..7   : the 8 512-blocks of the head's 4 pass-2 segments (pairs)
  slots 8..15  : remaining 8 blocks (pass-1 covers slots 0..15)
  slots 16..19 : the head's pass-3 segment blocks (duplicated copy)
Device computes, per pair, attention over 20*512=10240 slot-tokens;
host sums duplicated slots back into the 8192-token output.
"""

import sys
import os

for _p in ("/opt/trn_rl_repo", "/root/.axon_site/_ro/trn_rl_repo"):
    if os.path.isdir(_p) and _p not in sys.path:
        sys.path.insert(0, _p)

import numpy as np
from collections import deque
import ml_dtypes  # noqa: F401

import concourse.bass as bass
import concourse.tile as tile
from concourse import mybir
from concourse.bass_utils import run_bass_kernel_spmd

# ---------------------------------------------------------------- constants
B, NH, T, D = 2, 16, 8192, 64
BLK = 512                  # permutation block
NBLK = T // BLK            # 16
N_SLOTS = 20               # 16 + 4 duplicated pass-3 blocks
TS = N_SLOTS * BLK         # 10240 slot-tokens per pair
NCH = TS // 128            # 80 chunks of 128 tokens
N_CORES = 8
PAIRS_PER_CORE = 4

F32 = mybir.dt.float32
BF16 = mybir.dt.bfloat16
FP16 = mybir.dt.float16

# passes in slot-token space: (S, [segment token starts], init_or_accum)
PASSES = [
    (512, [BLK * i for i in range(16)], "init"),
    (1024, [1024 * j for j in range(4)], "accum"),
    (2048, [8192], "init"),
]


def _slot_map(h: int):
    p, a = h % 2, h % 4
    pass2 = [x for s in (p, p + 2, p + 4, p + 6) for x in (2 * s, 2 * s + 1)]
    rest = [b for b in range(16) if b not in pass2]
    p3 = [4 * a, 4 * a + 1, 4 * a + 2, 4 * a + 3]
    return pass2 + rest + p3  # 20 slots


# ------------------------------------------------------------- tile patch
def _patched_drain_and_barrier(self, tick_clock, wait_clock):
    # This walrus build rejects a CTRL Drain carrying >1 sync wait; split the
    # kernel-tail waits across one drain each.
    nc = self.nc
    di = nc.sync.drain()
    wait_clock.add_sem_waits(di.ins, tile.ScopedClock({None: tick_clock.global_clock}))
    si = di.ins.sync_info
    waits = list(si.on_wait)
    si.on_wait = waits[:1]
    proto = type(si)
    for w in waits[1:]:
        d2 = nc.sync.drain()
        d2.ins.sync_info = proto(on_wait=[w], on_update=[])
    nc.all_engine_barrier()
    popped = nc._tile_sem_poison_stack.pop()
    assert popped is self._sem_poison
    nc.clear_and_free_semaphores(list(self.sems.allocated().values()))
    nc.all_engine_barrier()


tile.TileContext._drain_and_barrier = _patched_drain_and_barrier


def _split_excess_waits(nc, max_waits=1):
    """This walrus build allows at most 2 sync waits per engine instruction
    (1 for CTRL/Drain). Move excess waits onto same-engine NOPs inserted
    immediately before the offending instruction."""
    proto = None
    for bbw in nc.bb_map.values():
        il = bbw.bb.instructions  # live list
        i = 0
        while i < len(il):
            inst = il[i]
            si = inst.sync_info
            limit = 1 if type(inst).__name__ == "InstDrain" else max_waits
            if si is not None and len(si.on_wait) > limit:
                waits = list(si.on_wait)
                if proto is None:
                    proto = type(si)
                keep = waits[len(waits) - limit:]
                over = waits[:len(waits) - limit]
                si.on_wait = keep
                # chunk the overflow onto nops (each nop takes max_waits)
                chunks = [over[j:j + max_waits]
                          for j in range(0, len(over), max_waits)]
                for ci, ch in enumerate(chunks):
                    bi = nc.engines[inst.engine].nop(nofuse=True)
                    nop_inst = bi.ins
                    # nop() appended nop_inst to the current bb; relocate it
                    for bb2 in nc.bb_map.values():
                        il2 = bb2.bb.instructions
                        if il2 and il2[-1] is nop_inst:
                            il2.pop()
                            break
                    nop_inst.sync_info = proto(on_wait=ch, on_update=[])
                    il.insert(i + ci, nop_inst)
                i += len(chunks)
            i += 1


# ------------------------------------------------------------ device program
_SIM_SAFE = [False]


def _block_groups(nc, pools, base, qt_t, kt_t, v1_t, otb_t, seg0, qb, run_q0):
    """Return (front, back) emitter pairs for one 512-query block.
    front = QK^T -> exp -> mask for one 2-bank psum group; back = its AV
    matmuls (and, on the block's last group, the po -> otb copy).
    Fronts/backs get software-pipelined by the caller so the PE always has
    several groups of QK^T work queued ahead of exp/mask-gated AVs.

    Each group is a list of sub-chunks (kc, sc_off, nq, q_off, diag_off):
    chunk kc's scores land at sc[:, sc_off:sc_off+nq] for queries
    [q0+q_off, q0+512); diag_off marks a 128-col causal-masked block."""
    sc_p, ex_p, po_p, rc_p = pools
    q0 = seg0 + 512 * qb
    n_full = 4 * qb
    total = n_full + 4
    po_t = po_p.tile([65, 512], F32, tag="po", name="po")
    kc0 = seg0 // 128

    gdefs = []
    for g0 in range(0, n_full, 3):
        cnt = min(3, n_full - g0)
        gdefs.append([(kc0 + g0 + j, 512 * j, 512, 0, None)
                      for j in range(cnt)])
    # diagonal wedge: chunks 4qb+m, queries from q0+128m; m=2/3 placed at
    # bank-aligned offsets (psum columns 896..1024 stay unwritten pad)
    gdefs.append([(kc0 + 4 * qb + 0, 0, 512, 0, 0),
                  (kc0 + 4 * qb + 1, 512, 384, 128, 512),
                  (kc0 + 4 * qb + 2, 1024, 256, 256, 1024),
                  (kc0 + 4 * qb + 3, 1280, 128, 384, 1280)])

    out = []
    state = {"cc": 0}

    def mk(subs):
        sc_t = sc_p.tile([128, 1536], F32, tag="sc", name="sc")
        ex_t = ex_p.tile([128, 1536], BF16, tag="ex", name="ex")
        # contiguous spans (the wedge has a pad gap at [896:1024])
        spans = []
        for off, end in sorted((off, off + nq) for _, off, nq, _, _ in subs):
            if spans and off <= spans[-1][1]:
                spans[-1][1] = max(spans[-1][1], end)
            else:
                spans.append([off, end])

        def front():
            for kc, off, nq, qo, _ in subs:
                nc.tensor.matmul(
                    sc_t[:, off:off + nq],
                    lhsT=kt_t[:, 128 * kc:128 * kc + 128],
                    rhs=qt_t[:, q0 + qo:q0 + 512],
                    start=True, stop=True,
                )
            if len(spans) == 1 or not _SIM_SAFE[0]:
                # pad gaps hold stale psum; exp of them is finite and unread
                nc.scalar.activation(
                    ex_t[:, spans[0][0]:spans[-1][1]],
                    sc_t[:, spans[0][0]:spans[-1][1]],
                    mybir.ActivationFunctionType.Exp, scale=0.125,
                )
            else:
                for a, b in spans:
                    nc.scalar.activation(
                        ex_t[:, a:b], sc_t[:, a:b],
                        mybir.ActivationFunctionType.Exp, scale=0.125,
                    )
            for _, off, nq, qo, do in subs:
                if do is None:
                    continue
                # keep exp where q_local - k_local >= 0 (causal, incl diag)
                nc.gpsimd.affine_select(
                    out=ex_t[:, do:do + 128],
                    in_=ex_t[:, do:do + 128],
                    compare_op=mybir.AluOpType.is_ge,
                    fill=0.0, base=0,
                    pattern=[[1, 128]], channel_multiplier=-1,
                )

        def back(last):
            for kc, off, nq, qo, _ in subs:
                nc.tensor.matmul(
                    po_t[:, qo:512],
                    lhsT=v1_t[:, 66 * kc:66 * kc + 65],
                    rhs=ex_t[:, off:off + nq],
                    start=(state["cc"] == 0),
                    stop=(state["cc"] == total - 1),
                )
                state["cc"] += 1
            if last:
                qc0 = q0 - run_q0
                nc.vector.tensor_copy(otb_t[0:65, qc0:qc0 + 512], po_t[:, :])

        return front, back

    for subs in gdefs:
        out.append(mk(subs))
    return out


def _emit_run(nc, pools, dma_pools, base, qt_t, kt_t, v1_t, acc_t,
              S, seg_list, mode, run_q0, n_tok, acc_ch0, backlog=None):
    """Emit all q-blocks of one contiguous pass-run, then launch the
    batch-normalize round trip: otb [66, n_tok] fp16 -> DRAM -> xbar back
    as [128, 66*K]. Returns a finisher (reciprocal + scaled accumulate into
    acc) that the caller schedules into a LATER phase's pipeline so its
    xbar wait never head-of-line blocks the DVE copy stream. `backlog`:
    deferred finishers to emit once this run's pipeline is a few groups
    deep."""
    sc_p, ex_p, po_p, rc_p = pools
    otb_p, otr_p, scro_p = dma_pools
    K = n_tok // 128
    otb_t = otb_p.tile([66, n_tok], FP16, tag="otb", name="otb")
    if _SIM_SAFE[0]:
        nc.vector.memset(otb_t[64:66, :], 0.0)  # pad row (64 rewritten)
    groups = []
    for seg0 in seg_list:
        for qb in range(S // 512):
            blk = _block_groups(nc, pools, base, qt_t, kt_t, v1_t, otb_t,
                                seg0, qb, run_q0)
            groups.extend(
                (front, back, gi == len(blk) - 1)
                for gi, (front, back) in enumerate(blk))
    # software pipeline: QK^T/exp of groups i+1..i+depth issue before the AV
    # of group i, so the PE never head-of-line blocks on exp->mask latency.
    # One deferred-normalize closure drains per group so the DVE copy stream
    # is never blocked by a lump of normalize work.
    depth = 2
    pend = []
    for gi, (front, back, last) in enumerate(groups):
        front()
        # one deferred piece per group, starting a couple of groups in
        if backlog and gi >= 2:
            backlog.popleft()()
        pend.append((back, last))
        if len(pend) > depth:
            b, l = pend.pop(0)
            b(l)
    for b, l in pend:
        b(l)
    scr_t = scro_p.tile([66, n_tok], FP16, tag="scrot", name="scrot")
    nc.sync.dma_start(out=scr_t[:, :], in_=otb_t[:, :])
    otr_t = otr_p.tile([128, 66 * K], FP16, tag="otr", name="otr")
    nc.sync.dma_start_transpose(
        otr_t[:, :], scr_t.rearrange("a (k b) -> (a k) b", b=128))

    # return fine-grained finisher closures: one reciprocal + normalize
    # pieces of two chunks each
    rc_t = rc_p.tile([128, K], F32, tag="rc", name="rc")
    otrv = otr_t.rearrange("p (d k) -> p d k", k=K)

    def recip():
        nc.vector.reciprocal(rc_t[:, :], otr_t[:, 64 * K:64 * K + K])

    def norm_piece(cb0, cb1):
        def go():
            for cb in range(cb0, cb1):
                cg = acc_ch0 + run_q0 // 128 + cb
                slc = acc_t[:, 64 * cg:64 * cg + 64]
                src = otrv[:, 0:64, cb]
                if mode == "init":
                    nc.vector.tensor_scalar_mul(slc, src, rc_t[:, cb:cb + 1])
                else:
                    nc.vector.scalar_tensor_tensor(
                        out=slc, in0=src, scalar=rc_t[:, cb:cb + 1], in1=slc,
                        op0=mybir.AluOpType.mult, op1=mybir.AluOpType.add,
                    )
        return go

    fins = [recip]
    for cb0 in range(0, K, 2):
        fins.append(norm_piece(cb0, min(cb0 + 2, K)))
    return fins


def build_program(n_pairs=PAIRS_PER_CORE, n_slots=N_SLOTS, passes=PASSES):
    ts = n_slots * BLK
    nch = ts // 128
    half_t = ts // 2          # tokens per half
    half_c = nch // 2         # chunks per half
    n_cpl = (n_pairs + 1) // 2
    nc = bass.Bass()
    qt_in = nc.declare_dram_parameter("QT", [n_cpl, 2, 128, ts // 2], BF16,
                                      isOutput=False)
    kt_in = nc.declare_dram_parameter("KT", [n_pairs, 2, 128, ts // 2], BF16,
                                      isOutput=False)
    v1_in = nc.declare_dram_parameter("V1", [n_pairs, 2, 128, 66 * (ts // 256)],
                                      BF16, isOutput=False)
    o_out = nc.declare_dram_parameter("Oc", [n_pairs, ts, D], F32, isOutput=True)

    # split passes into phase A (first half of slot-tokens) and B (second);
    # segments never cross the half boundary. Each pass's contiguous
    # segments inside a phase form one "run" (batch-normalized together).
    # Order inside a phase preserves init-before-accum.
    phase_runs = {0: [], 1: []}
    for (S, seg_starts, mode) in passes:
        for ph in (0, 1):
            segs = [s for s in seg_starts
                    if (0 if s + S <= half_t else 1) == ph]
            if not segs:
                continue
            segs.sort()
            local = [s - ph * half_t for s in segs]
            for a, b in zip(local, local[1:]):
                assert b == a + S, "run segments must be contiguous"
            assert local[0] % 128 == 0
            phase_runs[ph].append(
                (S, local, mode, local[0], len(local) * S))
    for ph in (0, 1):
        phase_runs[ph].sort(key=lambda x: 0 if x[2] == "init" else 1)

    n_couples = (n_pairs + 1) // 2
    with tile.TileContext(nc) as tc:
        with (
            tc.tile_pool(name="qk", bufs=2) as qk_p,  # qt/kt half tiles
            tc.tile_pool(name="v1", bufs=2) as v1_p,
            tc.tile_pool(name="acc", bufs=3) as acc_p,
            tc.tile_pool(name="ex", bufs=5) as ex_p,
            tc.tile_pool(name="rc", bufs=2) as rc_p,
            tc.tile_pool(name="otb", bufs=3) as otb_p,
            tc.tile_pool(name="otr", bufs=4) as otr_p,
            tc.tile_pool(name="sc", bufs=2, space="PSUM") as sc_p,
            tc.tile_pool(name="po", bufs=2, space="PSUM") as po_p,
            tc.tile_pool(name="scro", bufs=2, space="DRAM") as scro_p,
        ):
            pools = (sc_p, ex_p, po_p, rc_p)
            dma_pools = (otb_p, otr_p, scro_p)

            couples = [
                [p for p in (2 * c, 2 * c + 1) if p < n_pairs]
                for c in range(n_couples)
            ]
            qt_h, kt_h, v1_h = {}, {}, {}

            def prep(couple, hf):
                """Pure loads: pre-marshalled bf16 Q^T/K^T couple-half tiles
                and per-pair [V/3 | 1] layouts straight from HBM."""
                members = couples[couple]
                for pair in members:
                    # per-pair K^T, other partition half zeroed on the host:
                    # K=128 matmuls keep the PE HAM-warm (K=64 never warms)
                    kt_t = qk_p.tile([128, half_t], BF16, tag=f"kt{hf}",
                                     name=f"kt{hf}")
                    nc.sync.dma_start(out=kt_t[:, :], in_=kt_in[pair, hf])
                    kt_h[(pair, hf)] = kt_t
                qt_t = qk_p.tile([128, half_t], BF16, tag=f"qt{hf}",
                                 name=f"qt{hf}", bufs=1)
                nc.sync.dma_start(out=qt_t[:, :], in_=qt_in[couple, hf])
                qt_h[(couple, hf)] = qt_t
                for pi, pair in enumerate(members):
                    v1_t = v1_p.tile([128, 66 * half_c], BF16,
                                     tag=f"v1{hf}", name=f"v1{hf}")
                    v1_h[(pair, hf)] = v1_t
                    nc.sync.dma_start(out=v1_t[:, :], in_=v1_in[pair, hf])

            prep(0, 0)
            acc_ts = {}
            pending = deque()   # deferred fine-grained normalize closures
            due_outputs = []    # output DMA emitters awaiting finishers
            for couple in range(n_couples):
                members = couples[couple]
                for hf in range(2):
                    qt_t = qt_h[(couple, hf)]
                    for pi, pair in enumerate(members):
                        kt_t = kt_h[(pair, hf)]
                        if hf == 0:
                            acc_ts[pair] = acc_p.tile([128, 64 * nch], F32,
                                                      tag="acc", name="acc")
                        base = 64 * pi
                        backlog, pending = pending, deque()
                        outs, due_outputs = due_outputs, []
                        for (S, seg_list, mode, run_q0, n_tok) in phase_runs[hf]:
                            fins = _emit_run(
                                nc, pools, dma_pools, base, qt_t, kt_t,
                                v1_h[(pair, hf)], acc_ts[pair],
                                S, seg_list, mode, run_q0, n_tok,
                                acc_ch0=hf * half_c, backlog=backlog,
                            )
                            pending.extend(fins)
                        # anything the pipeline didn't drain, plus deferred
                        # output DMAs whose accs are now fully normalized
                        while backlog:
                            backlog.popleft()()
                        for oe in outs:
                            oe()
                        # interleave the next prep behind the first pair's
                        # compute so loads hide under current matmuls
                        if pi == 0 or len(members) == 1:
                            if hf == 0:
                                prep(couple, 1)
                            elif couple + 1 < n_couples:
                                prep(couple + 1, 0)

                def mk_out(pair, acc_t):
                    def go():
                        nc.sync.dma_start(
                            out=o_out[pair].rearrange("(c p) d -> p c d",
                                                      p=128),
                            in_=acc_t.rearrange("p (c d) -> p c d", d=64),
                        )
                    return go

                for pair in members:
                    due_outputs.append(mk_out(pair, acc_ts[pair]))
            while pending:
                pending.popleft()()
            for oe in due_outputs:
                oe()
    _split_excess_waits(nc)
    return nc


# ------------------------------------------------------------- host wrapper
_PROGRAM = None


def _get_program():
    global _PROGRAM
    if _PROGRAM is None:
        _PROGRAM = build_program()
    return _PROGRAM


_BF = ml_dtypes.bfloat16


def _marshal(qs, ks, vs):
    """[n_pairs, ts, 64] f32 triplet -> device input dict: bf16 transposed
    couple-half Q^T/K^T tiles and the per-pair strided [V/3 | 1] layout.
    Pure layout/dtype marshalling of the shard - no attention math."""
    n_pairs, ts, _ = qs.shape
    n_cpl = (n_pairs + 1) // 2
    half_t = ts // 2
    half_c = ts // 256

    def qt_of(arr):
        a = arr.astype(_BF).transpose(0, 2, 1)      # (pair, dd, t)
        a = a.reshape(n_cpl, 2, 64, 2, half_t)      # (cpl, pi, dd, hf, t)
        a = a.transpose(0, 3, 1, 2, 4)              # (cpl, hf, pi, dd, t)
        return np.ascontiguousarray(a.reshape(n_cpl, 2, 128, half_t))

    def kt_of(arr):
        a = arr.astype(_BF).transpose(0, 2, 1)          # (pair, dd, t)
        a = a.reshape(n_pairs, 64, 2, half_t).transpose(0, 2, 1, 3)
        out = np.zeros((n_pairs, 2, 128, half_t), dtype=_BF)
        for par in (0, 1):
            out[par::2, :, 64 * par:64 * par + 64] = a[par::2]
        return out

    v = vs.astype(np.float32) / 3.0
    v = v.reshape(n_pairs, 2, half_c, 128, 64)      # (pair, hf, c, p, dd)
    v1 = np.ones((n_pairs, 2, 128, half_c, 66), dtype=_BF)
    v1[..., :64] = v.transpose(0, 1, 3, 2, 4).astype(_BF)
    return {
        "QT": qt_of(qs),
        "KT": kt_of(ks),
        "V1": np.ascontiguousarray(v1.reshape(n_pairs, 2, 128, half_c * 66)),
    }


def _shard_inputs(Q, K, V):
    """-> list of 8 dicts with permuted+duplicated, marshalled per-core arrays."""
    in_maps = []
    for core in range(N_CORES):
        qs, ks, vs = [], [], []
        for pi in range(PAIRS_PER_CORE):
            flat = core * PAIRS_PER_CORE + pi
            b, h = flat // NH, flat % NH
            sm = _slot_map(h)
            for lst, src in ((qs, Q), (ks, K), (vs, V)):
                lst.append(
                    src[b, h].reshape(NBLK, BLK, D)[sm].reshape(TS, D)
                )
        in_maps.append(_marshal(np.stack(qs), np.stack(ks), np.stack(vs)))
    return in_maps


def _combine_outputs(results):
    out = np.zeros((B, NH, T, D), np.float32)
    for core in range(N_CORES):
        oc = results[core]["Oc"]  # [4, TS, D]
        for pi in range(PAIRS_PER_CORE):
            flat = core * PAIRS_PER_CORE + pi
            b, h = flat // NH, flat % NH
            sm = _slot_map(h)
            blocks = np.zeros((NBLK, BLK, D), np.float32)
            o = oc[pi].reshape(N_SLOTS, BLK, D)
            for slot, blk in enumerate(sm):
                blocks[blk] += o[slot]
            out[b, h] = blocks.reshape(T, D)
    return out


def kernel(Q, K, V):
    Q = np.asarray(Q, dtype=np.float32)
    K = np.asarray(K, dtype=np.float32)
    V = np.asarray(V, dtype=np.float32)
    nc = _get_program()
    in_maps = _shard_inputs(Q, K, V)
    res = run_bass_kernel_spmd(nc, in_maps, list(range(N_CORES)))
    return _combine_outputs(res.results)


if __name__ == "__main__":
    rng = np.random.default_rng(0)
    Q = rng.standard_normal((B, NH, T, D), dtype=np.float32)
    K = rng.standard_normal((B, NH, T, D), dtype=np.float32)
    V = rng.standard_normal((B, NH, T, D), dtype=np.float32)
    out = kernel(Q=Q, K=K, V=V)
    print("out", out.shape, out.dtype, float(np.abs(out).mean()))



# revision 5
# speedup vs baseline: 1.5780x; 1.5780x over previous
"""DilatedAttention Trainium2 kernel (telescoped schedule).

B=2, n=16 heads, T=8192, d=64. Three dilated passes (S,r) in
[(512,1),(1024,2),(2048,4)]; head h uses segments (h%r)+r*j; causal
softmax inside each segment; out = (p1+p2+p3)/3.

Key idea: the passes NEST. A pass-2 segment [A,B] satisfies
p2_out(A) == p1_out(A) and p2_num(B) = p1_num(B) + cross(A->B); the
pass-3 segment [W,X,Y,Z] telescopes the same way. So the device
computes, per 512-token block, one causal wedge plus a few cross-block
score panels, accumulating numerators in PSUM and snapshotting
(numerator | denominator) after each stage. The host divides, weights
(per-head parity) and scatters. 23 snapshots cover all 28 per-pass
block outputs; no score is computed twice.

Device details:
 - 16 blocks/pair (no duplication), 2 halves of 8 blocks.
 - QK^T uses PE row tiling: chunk 2j in array rows 0-63, chunk 2j+1 in
   rows 64-127 (Q^T duplicated in both partition halves) -> two K=64
   matmuls run concurrently, 2x effective QK^T rate.
 - wedge tiles: exact exp on ACT (+ gpsimd causal masks);
   cross tiles: Schraudolph int16-bitcast fp16 fast-exp on DVE
   (error dilutes into mixed numerators; validated 3.4e-3 rel err).
 - AV matmuls accumulate [V/3 | 1] so row 64 of each snapshot is the
   softmax denominator; normalization happens on the host.

Sharding: 32 (b,h) pairs -> 8 cores x 4 pairs.
"""

import sys
import os

for _p in ("/opt/trn_rl_repo", "/root/.axon_site/_ro/trn_rl_repo"):
    if os.path.isdir(_p) and _p not in sys.path:
        sys.path.insert(0, _p)

import numpy as np
from collections import deque
import ml_dtypes

import concourse.bass as bass
import concourse.tile as tile
from concourse import mybir
from concourse.bass_utils import run_bass_kernel_spmd

# ---------------------------------------------------------------- constants
B, NH, T, D = 2, 16, 8192, 64
BLK = 512
NBLK = 16
N_CORES = 8
PAIRS_PER_CORE = 4
HALF_T = 4096              # tokens per half
HALF_CH = 32               # 128-chunks per half
N_SNAPS = 23

F32 = mybir.dt.float32
BF16 = mybir.dt.bfloat16
FP16 = mybir.dt.float16
I16 = mybir.dt.int16

# fast-exp constants: exp(s/8) ~= bitcast_fp16(int16(s*A + B))
FE_A = 0.125 * 1024.0 * np.log2(np.e)          # 23.0830...
FE_C = 45.0
FE_B = 15.0 * 1024.0 - FE_C

# cross-stage spec: block-in-half -> list of (chunk_lo, chunk_hi) panels
CROSSES = {
    0: {0: [], 1: [(0, 4)], 2: [(0, 8)], 3: [(8, 12), (0, 8)],
        4: [], 5: [(16, 20)], 6: [], 7: [(24, 28)]},
    1: {0: [], 1: [(0, 4)], 2: [], 3: [], 4: [], 5: [], 6: [], 7: []},
}
N_SNAPS_HALF = {0: 14, 1: 9}

# snapshot -> slot (block within pair) and per-parity weights, device order
SNAP_SLOTS = [0, 1, 1, 2, 2, 3, 3, 3, 4, 5, 5, 6, 7, 7,
              8, 9, 9, 10, 11, 12, 13, 14, 15]


def _snap_weights(p):
    return np.array(
        [3 if p == 0 else 2,            # W wedge
         1, 2 if p == 0 else 1,         # X wedge, X cross
         1 if p == 0 else 2, 1,         # Y wedge, Y cross
         1, 0 if p == 0 else 1, 1,      # Z wedge, Z+YZ, Z+WX
         2, 1, 1,                       # S1
         2, 1, 1,                       # S2
         2, 1, 1,                       # S3
         1, 1, 1, 1, 1, 1],             # P1
        dtype=np.float32)


def _slot_map(h):
    """16 token-block indices in slot order: G3(4), 3 outside p2 segs(6),
    p1-only(6)."""
    p, a = h % 2, h % 4
    g3 = [4 * a + i for i in range(4)]
    segs = [p + 2 * j for j in range(4)]
    inside = 2 * a + p
    outside = sorted(s for s in segs if s != inside)
    oblk = [x for s in outside for x in (2 * s, 2 * s + 1)]
    used = set(g3) | set(oblk)
    p1only = [b for b in range(16) if b not in used]
    return g3 + oblk + p1only


# ------------------------------------------------------------- tile patches
def _patched_drain_and_barrier(self, tick_clock, wait_clock):
    # This walrus build rejects a CTRL Drain carrying >1 sync wait; split the
    # kernel-tail waits across one drain each.
    nc = self.nc
    di = nc.sync.drain()
    wait_clock.add_sem_waits(di.ins, tile.ScopedClock({None: tick_clock.global_clock}))
    si = di.ins.sync_info
    waits = list(si.on_wait)
    si.on_wait = waits[:1]
    proto = type(si)
    for w in waits[1:]:
        d2 = nc.sync.drain()
        d2.ins.sync_info = proto(on_wait=[w], on_update=[])
    nc.all_engine_barrier()
    popped = nc._tile_sem_poison_stack.pop()
    assert popped is self._sem_poison
    nc.clear_and_free_semaphores(list(self.sems.allocated().values()))
    nc.all_engine_barrier()


tile.TileContext._drain_and_barrier = _patched_drain_and_barrier


def _split_excess_waits(nc, max_waits=1):
    """This walrus build allows at most 2 sync waits per engine instruction
    (1 for CTRL/Drain). Move excess waits onto same-engine NOPs inserted
    immediately before the offending instruction."""
    proto = None
    for bbw in nc.bb_map.values():
        il = bbw.bb.instructions  # live list
        i = 0
        while i < len(il):
            inst = il[i]
            si = inst.sync_info
            limit = 1 if type(inst).__name__ == "InstDrain" else max_waits
            if si is not None and len(si.on_wait) > limit:
                waits = list(si.on_wait)
                if proto is None:
                    proto = type(si)
                keep = waits[len(waits) - limit:]
                over = waits[:len(waits) - limit]
                si.on_wait = keep
                chunks = [over[j:j + max_waits]
                          for j in range(0, len(over), max_waits)]
                for ci, ch in enumerate(chunks):
                    bi = nc.engines[inst.engine].nop(nofuse=True)
                    nop_inst = bi.ins
                    for bb2 in nc.bb_map.values():
                        il2 = bb2.bb.instructions
                        if il2 and il2[-1] is nop_inst:
                            il2.pop()
                            break
                    nop_inst.sync_info = proto(on_wait=ch, on_update=[])
                    il.insert(i + ci, nop_inst)
                i += len(chunks)
            i += 1


# ------------------------------------------------------------ device program
def _build_tiles(hf):
    """Tile list for one half: each tile = one sc PSUM tile with its QK
    subs, consumer kind, masks, AV subs and optional snapshot."""
    tiles = []
    for blk in range(8):
        c0 = 4 * blk
        q0 = 512 * blk
        # wedge: c0 @[0:512] qo0, c1 @[512:896] qo128, c3 @[896:1024] qo384,
        # c2 @[1024:1280] qo256  (bank-legal, gap-free packing)
        tiles.append(dict(
            kind="wedge", q0=q0,
            subs=[(c0 + 0, 0, 512, 0), (c0 + 1, 512, 384, 128),
                  (c0 + 3, 896, 128, 384), (c0 + 2, 1024, 256, 256)],
            width=1280, masks=[0, 512, 896, 1024],
            av_first=True, av_last=False, snap=True,
        ))
        stages = CROSSES[hf][blk]
        for si, (clo, chi) in enumerate(stages):
            cs = list(range(clo, chi))
            ntile = (len(cs) + 2) // 3
            for j0 in range(0, len(cs), 3):
                sub = [(c, 512 * i, 512, 0) for i, c in enumerate(cs[j0:j0 + 3])]
                tiles.append(dict(
                    kind="cross", q0=q0,
                    subs=sub, width=512 * len(sub), masks=[],
                    av_first=False, av_last=False,
                    snap=(j0 // 3 == ntile - 1),
                ))
        # mark stop on the block's very last AV matmul
        tiles[-1]["av_last"] = True
    return tiles


def build_program(n_pairs=PAIRS_PER_CORE):
    nc = bass.Bass()
    qt_in = nc.declare_dram_parameter("QT", [n_pairs, 2, 128, HALF_T], BF16,
                                      isOutput=False)
    kt_in = nc.declare_dram_parameter("KT", [n_pairs, 2, 128, HALF_T // 2], BF16,
                                      isOutput=False)
    v1_in = nc.declare_dram_parameter("V1", [n_pairs, 2, 128, 66 * HALF_CH],
                                      FP16, isOutput=False)
    o_out = nc.declare_dram_parameter("Oc", [n_pairs, 65, N_SNAPS * 512], FP16,
                                      isOutput=True)

    tiles_h = {0: _build_tiles(0), 1: _build_tiles(1)}

    with tile.TileContext(nc) as tc:
        with (
            tc.tile_pool(name="qt", bufs=2) as qt_p,
            tc.tile_pool(name="kt", bufs=2) as kt_p,
            tc.tile_pool(name="v1", bufs=2) as v1_p,
            tc.tile_pool(name="ex", bufs=4) as ex_p,
            tc.tile_pool(name="otb", bufs=2) as otb_p,
            tc.tile_pool(name="sc", bufs=2, space="PSUM") as sc_p,
            tc.tile_pool(name="po", bufs=2, space="PSUM") as po_p,
        ):
            tiles_sbuf = {}

            def prep(pair, hf):
                qt_t = qt_p.tile([128, HALF_T], BF16, tag="qt", name="qt")
                nc.sync.dma_start(out=qt_t[:, :], in_=qt_in[pair, hf])
                kt_t = kt_p.tile([128, HALF_T // 2], BF16, tag="kt", name="kt")
                nc.sync.dma_start(out=kt_t[:, :], in_=kt_in[pair, hf])
                v1_t = v1_p.tile([128, 66 * HALF_CH], FP16, tag="v1", name="v1")
                nc.sync.dma_start(out=v1_t[:, :], in_=v1_in[pair, hf])
                tiles_sbuf[(pair, hf)] = (qt_t, kt_t, v1_t)

            def emit_half(pair, hf, snap0, prefetch):
                qt_t, kt_t, v1_t = tiles_sbuf.pop((pair, hf))
                tiles = tiles_h[hf]
                n_snap = N_SNAPS_HALF[hf]
                otb_t = otb_p.tile([65, 512 * n_snap], FP16, tag="otb",
                                   name="otb")
                state = dict(po=None, snap=0)

                def front(t):
                    sc_t = sc_p.tile([128, 1536], F32, tag="sc", name="sc")
                    t["sc"] = sc_t
                    ex_t = ex_p.tile([128, 1536], FP16, tag="ex", name="ex")
                    t["ex"] = ex_t
                    q0 = t["q0"]
                    for (c, off, nq, qo) in t["subs"]:
                        r0 = 64 * (c % 2)
                        nc.tensor.matmul(
                            sc_t[:, off:off + nq],
                            lhsT=kt_t[r0:r0 + 64, 128 * (c // 2):128 * (c // 2) + 128],
                            rhs=qt_t[r0:r0 + 64, q0 + qo:q0 + qo + nq],
                            start=True, stop=True,
                        )
                    w = t["width"]
                    if t["kind"] == "wedge":
                        nc.scalar.activation(
                            ex_t[:, 0:w], sc_t[:, 0:w],
                            mybir.ActivationFunctionType.Exp, scale=0.125,
                        )
                        for do in t["masks"]:
                            nc.gpsimd.affine_select(
                                out=ex_t[:, do:do + 128],
                                in_=ex_t[:, do:do + 128],
                                compare_op=mybir.AluOpType.is_ge,
                                fill=0.0, base=0,
                                pattern=[[1, 128]], channel_multiplier=-1,
                            )
                    else:
                        nc.vector.tensor_scalar(
                            out=ex_t[:, 0:w].bitcast(I16),
                            in0=sc_t[:, 0:w],
                            scalar1=float(FE_A), scalar2=float(FE_B),
                            op0=mybir.AluOpType.mult,
                            op1=mybir.AluOpType.add,
                        )

                def back(t):
                    if t["av_first"]:
                        state["po"] = po_p.tile([65, 512], F32, tag="po",
                                                name="po")
                    po_t = state["po"]
                    ex_t = t["ex"]
                    subs = t["subs"]
                    for i, (c, off, nq, qo) in enumerate(subs):
                        nc.tensor.matmul(
                            po_t[:, qo:qo + nq],
                            lhsT=v1_t[:, 66 * c:66 * c + 65],
                            rhs=ex_t[:, off:off + nq],
                            start=(t["av_first"] and i == 0),
                            stop=(t["av_last"] and i == len(subs) - 1),
                        )
                    if t["snap"]:
                        s = state["snap"]
                        dst = otb_t[0:65, 512 * s:512 * s + 512]
                        if t["kind"] == "wedge":
                            nc.scalar.copy(dst, po_t[:, :])
                        else:
                            nc.vector.tensor_copy(dst, po_t[:, :])
                        state["snap"] += 1

                backs = deque()
                for i, t in enumerate(tiles):
                    front(t)
                    if i == 2 and prefetch is not None:
                        prep(*prefetch)
                    if len(backs) >= 2:
                        back(backs.popleft())
                    backs.append(t)
                while backs:
                    back(backs.popleft())
                assert state["snap"] == n_snap
                nc.sync.dma_start(
                    out=o_out[pair][:, 512 * snap0:512 * (snap0 + n_snap)],
                    in_=otb_t[:, :],
                )

            prep(0, 0)
            for pair in range(n_pairs):
                for hf in (0, 1):
                    nxt = (pair, 1) if hf == 0 else (
                        (pair + 1, 0) if pair + 1 < n_pairs else None)
                    emit_half(pair, hf, snap0=0 if hf == 0 else N_SNAPS_HALF[0],
                              prefetch=nxt)
    _split_excess_waits(nc)
    return nc


# ------------------------------------------------------------- host wrapper
_PROGRAM = None


def _get_program():
    global _PROGRAM
    if _PROGRAM is None:
        _PROGRAM = build_program()
    return _PROGRAM


_BF = ml_dtypes.bfloat16


def _marshal(qs, ks, vs):
    """[n_pairs, 16, 512, 64] f32 triplet (slot-ordered blocks) -> device
    input dict. Pure layout/dtype marshalling - no attention math."""
    n_pairs = qs.shape[0]
    q = qs.reshape(n_pairs, 2, HALF_T, D).transpose(0, 1, 3, 2).astype(_BF)
    qt = np.ascontiguousarray(np.concatenate([q, q], axis=2))  # dup d rows

    k = ks.reshape(n_pairs, 2, HALF_CH // 2, 2, 128, D)
    kt = np.ascontiguousarray(
        k.transpose(0, 1, 3, 5, 2, 4).reshape(n_pairs, 2, 128, HALF_T // 2)
        .astype(_BF))

    v = (vs.reshape(n_pairs, 2, HALF_CH, 128, D) / 3.0).astype(np.float16)
    v1 = np.ones((n_pairs, 2, HALF_CH, 128, 66), np.float16)
    v1[..., :64] = v
    v1 = np.ascontiguousarray(
        v1.transpose(0, 1, 3, 2, 4).reshape(n_pairs, 2, 128, HALF_CH * 66))
    return {"QT": qt, "KT": kt, "V1": v1}


def _shard_inputs(Q, K, V):
    in_maps = []
    for core in range(N_CORES):
        qs, ks, vs = [], [], []
        for pi in range(PAIRS_PER_CORE):
            flat = core * PAIRS_PER_CORE + pi
            b, h = flat // NH, flat % NH
            sm = _slot_map(h)
            qs.append(Q[b, h].reshape(NBLK, BLK, D)[sm])
            ks.append(K[b, h].reshape(NBLK, BLK, D)[sm])
            vs.append(V[b, h].reshape(NBLK, BLK, D)[sm])
        in_maps.append(_marshal(np.stack(qs), np.stack(ks), np.stack(vs)))
    return in_maps


_SNAP_SLOTS = np.array(SNAP_SLOTS)


def _combine_outputs(results):
    out = np.zeros((B, NH, T, D), np.float32)
    for core in range(N_CORES):
        oc_all = results[core]["Oc"]  # [4, 23, 65, 512] fp16
        for pi in range(PAIRS_PER_CORE):
            flat = core * PAIRS_PER_CORE + pi
            b, h = flat // NH, flat % NH
            sm = _slot_map(h)
            w = _snap_weights(h % 2)
            oc = oc_all[pi].astype(np.float32).reshape(65, N_SNAPS, 512)
            num = oc[0:64].transpose(1, 0, 2)          # [23, 64, 512]
            den = oc[64][:, None, :]                   # [23, 1, 512]
            snaps = (num / den) * w[:, None, None]     # [23, 64, 512]
            slotacc = np.zeros((NBLK, BLK, D), np.float32)
            np.add.at(slotacc, _SNAP_SLOTS, snaps.transpose(0, 2, 1))
            blocks = np.empty((NBLK, BLK, D), np.float32)
            blocks[sm] = slotacc
            out[b, h] = blocks.reshape(T, D)
    return out


def kernel(Q, K, V):
    Q = np.asarray(Q, dtype=np.float32)
    K = np.asarray(K, dtype=np.float32)
    V = np.asarray(V, dtype=np.float32)
    nc = _get_program()
    in_maps = _shard_inputs(Q, K, V)
    res = run_bass_kernel_spmd(nc, in_maps, list(range(N_CORES)))
    return _combine_outputs(res.results)


if __name__ == "__main__":
    rng = np.random.default_rng(0)
    Q = rng.standard_normal((B, NH, T, D), dtype=np.float32)
    K = rng.standard_normal((B, NH, T, D), dtype=np.float32)
    V = rng.standard_normal((B, NH, T, D), dtype=np.float32)
    out = kernel(Q=Q, K=K, V=V)
    print("out", out.shape, out.dtype, float(np.abs(out).mean()))


# revision 8
# speedup vs baseline: 1.6533x; 1.0477x over previous
"""DilatedAttention Trainium2 kernel (telescoped schedule).

B=2, n=16 heads, T=8192, d=64. Three dilated passes (S,r) in
[(512,1),(1024,2),(2048,4)]; head h uses segments (h%r)+r*j; causal
softmax inside each segment; out = (p1+p2+p3)/3.

Key idea: the passes NEST. A pass-2 segment [A,B] satisfies
p2_out(A) == p1_out(A) and p2_num(B) = p1_num(B) + cross(A->B); the
pass-3 segment [W,X,Y,Z] telescopes the same way. So the device
computes, per 512-token block, one causal wedge plus a few cross-block
score panels, accumulating numerators in PSUM and snapshotting
(numerator | denominator) after each stage. The host divides, weights
(per-head parity) and scatters. 23 snapshots cover all 28 per-pass
block outputs; no score is computed twice.

Device details:
 - 16 blocks/pair (no duplication), 2 halves of 8 blocks.
 - QK^T uses PE row tiling: chunk 2j in array rows 0-63, chunk 2j+1 in
   rows 64-127 (Q^T duplicated in both partition halves) -> two K=64
   matmuls run concurrently, 2x effective QK^T rate.
 - wedge tiles: exact exp on ACT (+ gpsimd causal masks);
   cross tiles: Schraudolph int16-bitcast fp16 fast-exp on DVE
   (error dilutes into mixed numerators; validated 3.4e-3 rel err).
 - AV matmuls accumulate [V/3 | 1] so row 64 of each snapshot is the
   softmax denominator; normalization happens on the host.

Sharding: 32 (b,h) pairs -> 8 cores x 4 pairs.
"""

import sys
import os

for _p in ("/opt/trn_rl_repo", "/root/.axon_site/_ro/trn_rl_repo"):
    if os.path.isdir(_p) and _p not in sys.path:
        sys.path.insert(0, _p)

import numpy as np
from collections import deque
import ml_dtypes

import concourse.bass as bass
import concourse.tile as tile
from concourse import mybir
from concourse.bass_utils import run_bass_kernel_spmd

# ---------------------------------------------------------------- constants
B, NH, T, D = 2, 16, 8192, 64
BLK = 512
NBLK = 16
N_CORES = 8
PAIRS_PER_CORE = 4
HALF_T = 4096              # tokens per half
HALF_CH = 32               # 128-chunks per half
N_SNAPS = 23

F32 = mybir.dt.float32
BF16 = mybir.dt.bfloat16
FP16 = mybir.dt.float16
I16 = mybir.dt.int16

# fast-exp constants: exp(s/8) ~= bitcast_fp16(int16(s*A + B))
FE_A = 0.125 * 1024.0 * np.log2(np.e)          # 23.0830...
FE_C = 45.0
FE_B = 15.0 * 1024.0 - FE_C

# cross-stage spec: block-in-half -> list of (chunk_lo, chunk_hi) panels
CROSSES = {
    0: {0: [], 1: [(0, 4)], 2: [(0, 8)], 3: [(8, 12), (0, 8)],
        4: [], 5: [(16, 20)], 6: [], 7: [(24, 28)]},
    1: {0: [], 1: [(0, 4)], 2: [], 3: [], 4: [], 5: [], 6: [], 7: []},
}
N_SNAPS_HALF = {0: 14, 1: 9}

# snapshot -> slot (block within pair) and per-parity weights, device order
SNAP_SLOTS = [0, 1, 1, 2, 2, 3, 3, 3, 4, 5, 5, 6, 7, 7,
              8, 9, 9, 10, 11, 12, 13, 14, 15]


def _snap_weights(p):
    return np.array(
        [3 if p == 0 else 2,            # W wedge
         1, 2 if p == 0 else 1,         # X wedge, X cross
         1 if p == 0 else 2, 1,         # Y wedge, Y cross
         1, 0 if p == 0 else 1, 1,      # Z wedge, Z+YZ, Z+WX
         2, 1, 1,                       # S1
         2, 1, 1,                       # S2
         2, 1, 1,                       # S3
         1, 1, 1, 1, 1, 1],             # P1
        dtype=np.float32)


def _slot_map(h):
    """16 token-block indices in slot order: G3(4), 3 outside p2 segs(6),
    p1-only(6)."""
    p, a = h % 2, h % 4
    g3 = [4 * a + i for i in range(4)]
    segs = [p + 2 * j for j in range(4)]
    inside = 2 * a + p
    outside = sorted(s for s in segs if s != inside)
    oblk = [x for s in outside for x in (2 * s, 2 * s + 1)]
    used = set(g3) | set(oblk)
    p1only = [b for b in range(16) if b not in used]
    return g3 + oblk + p1only


# ------------------------------------------------------------- tile patches
def _patched_drain_and_barrier(self, tick_clock, wait_clock):
    # This walrus build rejects a CTRL Drain carrying >1 sync wait; split the
    # kernel-tail waits across one drain each.
    nc = self.nc
    di = nc.sync.drain()
    wait_clock.add_sem_waits(di.ins, tile.ScopedClock({None: tick_clock.global_clock}))
    si = di.ins.sync_info
    waits = list(si.on_wait)
    si.on_wait = waits[:1]
    proto = type(si)
    for w in waits[1:]:
        d2 = nc.sync.drain()
        d2.ins.sync_info = proto(on_wait=[w], on_update=[])
    nc.all_engine_barrier()
    popped = nc._tile_sem_poison_stack.pop()
    assert popped is self._sem_poison
    nc.clear_and_free_semaphores(list(self.sems.allocated().values()))
    nc.all_engine_barrier()


tile.TileContext._drain_and_barrier = _patched_drain_and_barrier


def _split_excess_waits(nc, max_waits=1):
    """This walrus build allows at most 2 sync waits per engine instruction
    (1 for CTRL/Drain). Move excess waits onto same-engine NOPs inserted
    immediately before the offending instruction."""
    proto = None
    for bbw in nc.bb_map.values():
        il = bbw.bb.instructions  # live list
        i = 0
        while i < len(il):
            inst = il[i]
            si = inst.sync_info
            limit = 1 if type(inst).__name__ == "InstDrain" else max_waits
            if si is not None and len(si.on_wait) > limit:
                waits = list(si.on_wait)
                if proto is None:
                    proto = type(si)
                keep = waits[len(waits) - limit:]
                over = waits[:len(waits) - limit]
                si.on_wait = keep
                chunks = [over[j:j + max_waits]
                          for j in range(0, len(over), max_waits)]
                for ci, ch in enumerate(chunks):
                    bi = nc.engines[inst.engine].nop(nofuse=True)
                    nop_inst = bi.ins
                    for bb2 in nc.bb_map.values():
                        il2 = bb2.bb.instructions
                        if il2 and il2[-1] is nop_inst:
                            il2.pop()
                            break
                    nop_inst.sync_info = proto(on_wait=ch, on_update=[])
                    il.insert(i + ci, nop_inst)
                i += len(chunks)
            i += 1


# ------------------------------------------------------------ device program
def _build_tiles(hf):
    """Tile list for one half: each tile = one sc PSUM tile with its QK
    subs, consumer kind, masks, AV subs and optional snapshot."""
    tiles = []
    for blk in range(8):
        c0 = 4 * blk
        q0 = 512 * blk
        # wedge: c0 @[0:512] qo0, c1 @[512:896] qo128, c3 @[896:1024] qo384,
        # c2 @[1024:1280] qo256  (bank-legal, gap-free packing)
        tiles.append(dict(
            kind="wedge", q0=q0,
            subs=[(c0 + 0, 0, 512, 0), (c0 + 1, 512, 384, 128),
                  (c0 + 3, 896, 128, 384), (c0 + 2, 1024, 256, 256)],
            width=1280, masks=[0, 512, 896, 1024],
            av_first=True, av_last=False, snap=True,
        ))
        stages = CROSSES[hf][blk]
        for si, (clo, chi) in enumerate(stages):
            cs = list(range(clo, chi))
            ntile = (len(cs) + 2) // 3
            for j0 in range(0, len(cs), 3):
                sub = [(c, 512 * i, 512, 0) for i, c in enumerate(cs[j0:j0 + 3])]
                tiles.append(dict(
                    kind="cross", q0=q0,
                    subs=sub, width=512 * len(sub), masks=[],
                    av_first=False, av_last=False,
                    snap=(j0 // 3 == ntile - 1),
                ))
        # mark stop on the block's very last AV matmul
        tiles[-1]["av_last"] = True
    return tiles


def build_program(n_pairs=PAIRS_PER_CORE):
    nc = bass.Bass()
    qt_in = nc.declare_dram_parameter("QT", [n_pairs, 2, 128, HALF_T], BF16,
                                      isOutput=False)
    kt_in = nc.declare_dram_parameter("KT", [n_pairs, 2, 128, HALF_T // 2], BF16,
                                      isOutput=False)
    v1_in = nc.declare_dram_parameter("V1", [n_pairs, 2, 128, 66 * HALF_CH],
                                      FP16, isOutput=False)
    o_out = nc.declare_dram_parameter("Oc", [n_pairs, 65, N_SNAPS * 512], FP16,
                                      isOutput=True)

    tiles_h = {0: _build_tiles(0), 1: _build_tiles(1)}

    # piece split: blocks 0-1 / chunks 0-7 load first so compute starts early
    QT_SPLIT = 1024          # qt cols (tokens)
    KT_SPLIT = 512           # kt cols (= chunks 0-7)
    V1_SPLIT = 66 * 8        # v1 cols (= chunks 0-7)

    with tile.TileContext(nc) as tc:
        with (
            tc.tile_pool(name="qt0", bufs=2) as qt0_p,
            tc.tile_pool(name="qt1", bufs=2) as qt1_p,
            tc.tile_pool(name="kt0", bufs=2) as kt0_p,
            tc.tile_pool(name="kt1", bufs=2) as kt1_p,
            tc.tile_pool(name="v10", bufs=2) as v10_p,
            tc.tile_pool(name="v11", bufs=2) as v11_p,
            tc.tile_pool(name="ex", bufs=4) as ex_p,
            tc.tile_pool(name="otb", bufs=2) as otb_p,
            tc.tile_pool(name="sc", bufs=2, space="PSUM") as sc_p,
            tc.tile_pool(name="po", bufs=2, space="PSUM") as po_p,
        ):
            tiles_sbuf = {}

            def prep(pair, hf):
                qt0 = qt0_p.tile([128, QT_SPLIT], BF16, tag="qt0", name="qt0")
                nc.sync.dma_start(out=qt0[:, :], in_=qt_in[pair, hf][:, 0:QT_SPLIT])
                kt0 = kt0_p.tile([128, KT_SPLIT], BF16, tag="kt0", name="kt0")
                nc.sync.dma_start(out=kt0[:, :], in_=kt_in[pair, hf][:, 0:KT_SPLIT])
                v10 = v10_p.tile([128, V1_SPLIT], FP16, tag="v10", name="v10")
                nc.sync.dma_start(out=v10[:, :], in_=v1_in[pair, hf][:, 0:V1_SPLIT])
                qt1 = qt1_p.tile([128, HALF_T - QT_SPLIT], BF16, tag="qt1",
                                 name="qt1")
                nc.sync.dma_start(out=qt1[:, :], in_=qt_in[pair, hf][:, QT_SPLIT:])
                kt1 = kt1_p.tile([128, HALF_T // 2 - KT_SPLIT], BF16, tag="kt1",
                                 name="kt1")
                nc.sync.dma_start(out=kt1[:, :], in_=kt_in[pair, hf][:, KT_SPLIT:])
                v11 = v11_p.tile([128, 66 * HALF_CH - V1_SPLIT], FP16, tag="v11",
                                 name="v11")
                nc.sync.dma_start(out=v11[:, :], in_=v1_in[pair, hf][:, V1_SPLIT:])
                tiles_sbuf[(pair, hf)] = (qt0, qt1, kt0, kt1, v10, v11)

            def emit_half(pair, hf, snap0, prefetch):
                qt0, qt1, kt0, kt1, v10, v11 = tiles_sbuf.pop((pair, hf))

                def qt_ap(r0, a, b):
                    if b <= QT_SPLIT:
                        return qt0[r0:r0 + 64, a:b]
                    return qt1[r0:r0 + 64, a - QT_SPLIT:b - QT_SPLIT]

                def kt_ap(c):
                    r0, col = 64 * (c % 2), 128 * (c // 2)
                    if col < KT_SPLIT:
                        return kt0[r0:r0 + 64, col:col + 128]
                    return kt1[r0:r0 + 64, col - KT_SPLIT:col - KT_SPLIT + 128]

                def v1_ap(c):
                    col = 66 * c
                    if col < V1_SPLIT:
                        return v10[:, col:col + 65]
                    return v11[:, col - V1_SPLIT:col - V1_SPLIT + 65]
                tiles = tiles_h[hf]
                n_snap = N_SNAPS_HALF[hf]
                otb_t = otb_p.tile([65, 512 * n_snap], FP16, tag="otb",
                                   name="otb")
                state = dict(po=None, snap=0)

                def front(t):
                    sc_t = sc_p.tile([128, 1536], F32, tag="sc", name="sc")
                    t["sc"] = sc_t
                    ex_t = ex_p.tile([128, 1536], FP16, tag="ex", name="ex")
                    t["ex"] = ex_t
                    q0 = t["q0"]
                    for (c, off, nq, qo) in t["subs"]:
                        r0 = 64 * (c % 2)
                        nc.tensor.matmul(
                            sc_t[:, off:off + nq],
                            lhsT=kt_ap(c),
                            rhs=qt_ap(r0, q0 + qo, q0 + qo + nq),
                            start=True, stop=True,
                        )
                    w = t["width"]
                    if t["kind"] == "wedge":
                        nc.scalar.activation(
                            ex_t[:, 0:w], sc_t[:, 0:w],
                            mybir.ActivationFunctionType.Exp, scale=0.125,
                        )
                        for do in t["masks"]:
                            nc.gpsimd.affine_select(
                                out=ex_t[:, do:do + 128],
                                in_=ex_t[:, do:do + 128],
                                compare_op=mybir.AluOpType.is_ge,
                                fill=0.0, base=0,
                                pattern=[[1, 128]], channel_multiplier=-1,
                            )
                    else:
                        nc.vector.tensor_scalar(
                            out=ex_t[:, 0:w].bitcast(I16),
                            in0=sc_t[:, 0:w],
                            scalar1=float(FE_A), scalar2=float(FE_B),
                            op0=mybir.AluOpType.mult,
                            op1=mybir.AluOpType.add,
                        )

                def back(t):
                    if t["av_first"]:
                        state["po"] = po_p.tile([65, 512], F32, tag="po",
                                                name="po")
                    po_t = state["po"]
                    ex_t = t["ex"]
                    subs = t["subs"]
                    for i, (c, off, nq, qo) in enumerate(subs):
                        nc.tensor.matmul(
                            po_t[:, qo:qo + nq],
                            lhsT=v1_ap(c),
                            rhs=ex_t[:, off:off + nq],
                            start=(t["av_first"] and i == 0),
                            stop=(t["av_last"] and i == len(subs) - 1),
                        )
                    if t["snap"]:
                        s = state["snap"]
                        dst = otb_t[0:65, 512 * s:512 * s + 512]
                        # engine balance: half0 wedge snaps on ACT (DVE busy
                        # with cross fast-exp there); everything else on DVE
                        if t["kind"] == "wedge" and hf == 0:
                            nc.scalar.copy(dst, po_t[:, :])
                        else:
                            nc.vector.tensor_copy(dst, po_t[:, :])
                        state["snap"] += 1

                backs = deque()
                for i, t in enumerate(tiles):
                    front(t)
                    if i == 2 and prefetch is not None:
                        prep(*prefetch)
                    if len(backs) >= 2:
                        back(backs.popleft())
                    backs.append(t)
                while backs:
                    back(backs.popleft())
                assert state["snap"] == n_snap
                nc.sync.dma_start(
                    out=o_out[pair][:, 512 * snap0:512 * (snap0 + n_snap)],
                    in_=otb_t[:, :],
                )

            prep(0, 0)
            for pair in range(n_pairs):
                for hf in (0, 1):
                    nxt = (pair, 1) if hf == 0 else (
                        (pair + 1, 0) if pair + 1 < n_pairs else None)
                    emit_half(pair, hf, snap0=0 if hf == 0 else N_SNAPS_HALF[0],
                              prefetch=nxt)
    _split_excess_waits(nc)
    return nc


# ------------------------------------------------------------- host wrapper
_PROGRAM = None


def _get_program():
    global _PROGRAM
    if _PROGRAM is None:
        _PROGRAM = build_program()
    return _PROGRAM


_BF = ml_dtypes.bfloat16


def _marshal(qs, ks, vs):
    """[n_pairs, 16, 512, 64] f32 triplet (slot-ordered blocks) -> device
    input dict. Pure layout/dtype marshalling - no attention math."""
    n_pairs = qs.shape[0]
    q = qs.reshape(n_pairs, 2, HALF_T, D).transpose(0, 1, 3, 2).astype(_BF)
    qt = np.ascontiguousarray(np.concatenate([q, q], axis=2))  # dup d rows

    k = ks.reshape(n_pairs, 2, HALF_CH // 2, 2, 128, D)
    kt = np.ascontiguousarray(
        k.transpose(0, 1, 3, 5, 2, 4).reshape(n_pairs, 2, 128, HALF_T // 2)
        .astype(_BF))

    v = (vs.reshape(n_pairs, 2, HALF_CH, 128, D) / 3.0).astype(np.float16)
    v1 = np.ones((n_pairs, 2, HALF_CH, 128, 66), np.float16)
    v1[..., :64] = v
    v1 = np.ascontiguousarray(
        v1.transpose(0, 1, 3, 2, 4).reshape(n_pairs, 2, 128, HALF_CH * 66))
    return {"QT": qt, "KT": kt, "V1": v1}


def _shard_inputs(Q, K, V):
    in_maps = []
    for core in range(N_CORES):
        qs, ks, vs = [], [], []
        for pi in range(PAIRS_PER_CORE):
            flat = core * PAIRS_PER_CORE + pi
            b, h = flat // NH, flat % NH
            sm = _slot_map(h)
            qs.append(Q[b, h].reshape(NBLK, BLK, D)[sm])
            ks.append(K[b, h].reshape(NBLK, BLK, D)[sm])
            vs.append(V[b, h].reshape(NBLK, BLK, D)[sm])
        in_maps.append(_marshal(np.stack(qs), np.stack(ks), np.stack(vs)))
    return in_maps


_SNAP_SLOTS = np.array(SNAP_SLOTS)


def _combine_outputs(results):
    out = np.zeros((B, NH, T, D), np.float32)
    for core in range(N_CORES):
        oc_all = results[core]["Oc"]  # [4, 23, 65, 512] fp16
        for pi in range(PAIRS_PER_CORE):
            flat = core * PAIRS_PER_CORE + pi
            b, h = flat // NH, flat % NH
            sm = _slot_map(h)
            w = _snap_weights(h % 2)
            oc = oc_all[pi].astype(np.float32).reshape(65, N_SNAPS, 512)
            num = oc[0:64].transpose(1, 0, 2)          # [23, 64, 512]
            den = oc[64][:, None, :]                   # [23, 1, 512]
            snaps = (num / den) * w[:, None, None]     # [23, 64, 512]
            slotacc = np.zeros((NBLK, BLK, D), np.float32)
            np.add.at(slotacc, _SNAP_SLOTS, snaps.transpose(0, 2, 1))
            blocks = np.empty((NBLK, BLK, D), np.float32)
            blocks[sm] = slotacc
            out[b, h] = blocks.reshape(T, D)
    return out


def kernel(Q, K, V):
    Q = np.asarray(Q, dtype=np.float32)
    K = np.asarray(K, dtype=np.float32)
    V = np.asarray(V, dtype=np.float32)
    nc = _get_program()
    in_maps = _shard_inputs(Q, K, V)
    res = run_bass_kernel_spmd(nc, in_maps, list(range(N_CORES)))
    return _combine_outputs(res.results)


if __name__ == "__main__":
    rng = np.random.default_rng(0)
    Q = rng.standard_normal((B, NH, T, D), dtype=np.float32)
    K = rng.standard_normal((B, NH, T, D), dtype=np.float32)
    V = rng.standard_normal((B, NH, T, D), dtype=np.float32)
    out = kernel(Q=Q, K=K, V=V)
    print("out", out.shape, out.dtype, float(np.abs(out).mean()))


# revision 11
# speedup vs baseline: 1.7490x; 1.0579x over previous
"""DilatedAttention Trainium2 kernel (telescoped schedule).

B=2, n=16 heads, T=8192, d=64. Three dilated passes (S,r) in
[(512,1),(1024,2),(2048,4)]; head h uses segments (h%r)+r*j; causal
softmax inside each segment; out = (p1+p2+p3)/3.

Key idea: the passes NEST. A pass-2 segment [A,B] satisfies
p2_out(A) == p1_out(A) and p2_num(B) = p1_num(B) + cross(A->B); the
pass-3 segment [W,X,Y,Z] telescopes the same way. So the device
computes, per 512-token block, one causal wedge plus a few cross-block
score panels, accumulating numerators in PSUM and snapshotting
(numerator | denominator) after each stage. The host divides, weights
(per-head parity) and scatters. 23 snapshots cover all 28 per-pass
block outputs; no score is computed twice.

Device details:
 - 16 blocks/pair (no duplication), 2 halves of 8 blocks.
 - QK^T uses PE row tiling: chunk 2j in array rows 0-63, chunk 2j+1 in
   rows 64-127 (Q^T duplicated in both partition halves) -> two K=64
   matmuls run concurrently, 2x effective QK^T rate.
 - wedge tiles: exact exp on ACT (+ gpsimd causal masks);
   cross tiles: Schraudolph int16-bitcast fp16 fast-exp on DVE
   (error dilutes into mixed numerators; validated 3.4e-3 rel err).
 - AV matmuls accumulate [V/3 | 1] so row 64 of each snapshot is the
   softmax denominator; normalization happens on the host.

Sharding: 32 (b,h) pairs -> 8 cores x 4 pairs.
"""

import sys
import os

for _p in ("/opt/trn_rl_repo", "/root/.axon_site/_ro/trn_rl_repo"):
    if os.path.isdir(_p) and _p not in sys.path:
        sys.path.insert(0, _p)

import numpy as np
from collections import deque
import ml_dtypes

import concourse.bass as bass
import concourse.tile as tile
from concourse import mybir
from concourse.bass_utils import run_bass_kernel_spmd

# ---------------------------------------------------------------- constants
B, NH, T, D = 2, 16, 8192, 64
BLK = 512
NBLK = 16
N_CORES = 8
PAIRS_PER_CORE = 4
HALF_T = 4096              # tokens per half
HALF_CH = 32               # 128-chunks per half
N_SNAPS = 23

F32 = mybir.dt.float32
BF16 = mybir.dt.bfloat16
FP16 = mybir.dt.float16
I16 = mybir.dt.int16

# fast-exp constants: exp(s/8) ~= bitcast_fp16(int16(s*A + B))
FE_A = 0.125 * 1024.0 * np.log2(np.e)          # 23.0830...
FE_C = 45.0
FE_B = 15.0 * 1024.0 - FE_C

# cross-stage spec: block-in-half -> list of (chunk_lo, chunk_hi) panels
CROSSES = {
    0: {0: [], 1: [(0, 4)], 2: [(0, 8)], 3: [(8, 12), (0, 8)],
        4: [], 5: [(16, 20)], 6: [], 7: [(24, 28)]},
    1: {0: [], 1: [(0, 4)], 2: [], 3: [], 4: [], 5: [], 6: [], 7: []},
}
N_SNAPS_HALF = {0: 14, 1: 9}

# snapshot -> slot (block within pair) and per-parity weights, device order
SNAP_SLOTS = [0, 1, 1, 2, 2, 3, 3, 3, 4, 5, 5, 6, 7, 7,
              8, 9, 9, 10, 11, 12, 13, 14, 15]


def _snap_weights(p):
    return np.array(
        [3 if p == 0 else 2,            # W wedge
         1, 2 if p == 0 else 1,         # X wedge, X cross
         1 if p == 0 else 2, 1,         # Y wedge, Y cross
         1, 0 if p == 0 else 1, 1,      # Z wedge, Z+YZ, Z+WX
         2, 1, 1,                       # S1
         2, 1, 1,                       # S2
         2, 1, 1,                       # S3
         1, 1, 1, 1, 1, 1],             # P1
        dtype=np.float32)


def _slot_map(h):
    """16 token-block indices in slot order: G3(4), 3 outside p2 segs(6),
    p1-only(6)."""
    p, a = h % 2, h % 4
    g3 = [4 * a + i for i in range(4)]
    segs = [p + 2 * j for j in range(4)]
    inside = 2 * a + p
    outside = sorted(s for s in segs if s != inside)
    oblk = [x for s in outside for x in (2 * s, 2 * s + 1)]
    used = set(g3) | set(oblk)
    p1only = [b for b in range(16) if b not in used]
    return g3 + oblk + p1only


# ------------------------------------------------------------- tile patches
def _patched_drain_and_barrier(self, tick_clock, wait_clock):
    # This walrus build rejects a CTRL Drain carrying >1 sync wait; split the
    # kernel-tail waits across one drain each.
    nc = self.nc
    di = nc.sync.drain()
    wait_clock.add_sem_waits(di.ins, tile.ScopedClock({None: tick_clock.global_clock}))
    si = di.ins.sync_info
    waits = list(si.on_wait)
    si.on_wait = waits[:1]
    proto = type(si)
    for w in waits[1:]:
        d2 = nc.sync.drain()
        d2.ins.sync_info = proto(on_wait=[w], on_update=[])
    nc.all_engine_barrier()
    popped = nc._tile_sem_poison_stack.pop()
    assert popped is self._sem_poison
    nc.clear_and_free_semaphores(list(self.sems.allocated().values()))
    nc.all_engine_barrier()


tile.TileContext._drain_and_barrier = _patched_drain_and_barrier


def _split_excess_waits(nc, max_waits=1):
    """This walrus build allows at most 2 sync waits per engine instruction
    (1 for CTRL/Drain). Move excess waits onto same-engine NOPs inserted
    immediately before the offending instruction."""
    proto = None
    for bbw in nc.bb_map.values():
        il = bbw.bb.instructions  # live list
        i = 0
        while i < len(il):
            inst = il[i]
            si = inst.sync_info
            limit = 1 if type(inst).__name__ == "InstDrain" else max_waits
            if si is not None and len(si.on_wait) > limit:
                waits = list(si.on_wait)
                if proto is None:
                    proto = type(si)
                keep = waits[len(waits) - limit:]
                over = waits[:len(waits) - limit]
                si.on_wait = keep
                chunks = [over[j:j + max_waits]
                          for j in range(0, len(over), max_waits)]
                for ci, ch in enumerate(chunks):
                    bi = nc.engines[inst.engine].nop(nofuse=True)
                    nop_inst = bi.ins
                    for bb2 in nc.bb_map.values():
                        il2 = bb2.bb.instructions
                        if il2 and il2[-1] is nop_inst:
                            il2.pop()
                            break
                    nop_inst.sync_info = proto(on_wait=ch, on_update=[])
                    il.insert(i + ci, nop_inst)
                i += len(chunks)
            i += 1


# ------------------------------------------------------------ device program
def _build_tiles(hf):
    """Tile list for one half: each tile = one sc PSUM tile with its QK
    subs, consumer kind, masks, AV subs and optional snapshot."""
    tiles = []
    for blk in range(8):
        c0 = 4 * blk
        q0 = 512 * blk
        # wedge: c0 @[0:512] qo0, c1 @[512:896] qo128, c3 @[896:1024] qo384,
        # c2 @[1024:1280] qo256  (bank-legal, gap-free packing)
        # engine balance: half1 is ACT-bound (few crosses) -> last P1 wedge
        # uses DVE fast-exp there
        tiles.append(dict(
            kind="wedge", q0=q0,
            subs=[(c0 + 0, 0, 512, 0), (c0 + 1, 512, 384, 128),
                  (c0 + 3, 896, 128, 384), (c0 + 2, 1024, 256, 256)],
            width=1280, masks=[0, 512, 896, 1024],
            av_first=True, av_last=False, snap=True,
            fe=(hf == 1 and blk == 7),
        ))
        stages = CROSSES[hf][blk]
        for si, (clo, chi) in enumerate(stages):
            cs = list(range(clo, chi))
            ntile = (len(cs) + 2) // 3
            for j0 in range(0, len(cs), 3):
                ti = j0 // 3
                tiles.append(dict(
                    kind="cross", q0=q0,
                    subs=[(c, 512 * i, 512, 0)
                          for i, c in enumerate(cs[j0:j0 + 3])],
                    width=512 * len(cs[j0:j0 + 3]), masks=[],
                    av_first=False, av_last=False,
                    snap=(ti == ntile - 1),
                    # DVE-burst relief: middle tile of 3-tile stages on ACT
                    fe=not (ntile >= 3 and ti == 1),
                ))
        # mark stop on the block's very last AV matmul
        tiles[-1]["av_last"] = True
    return tiles


def build_program(n_pairs=PAIRS_PER_CORE):
    nc = bass.Bass()
    qt_in = nc.declare_dram_parameter("QT", [n_pairs, 2, 128, HALF_T], BF16,
                                      isOutput=False)
    kt_in = nc.declare_dram_parameter("KT", [n_pairs, 2, 128, HALF_T // 2], BF16,
                                      isOutput=False)
    v1_in = nc.declare_dram_parameter("V1", [n_pairs, 2, 128, 66 * HALF_CH],
                                      FP16, isOutput=False)
    o_out = nc.declare_dram_parameter("Oc", [n_pairs, 65, N_SNAPS * 512], FP16,
                                      isOutput=True)

    tiles_h = {0: _build_tiles(0), 1: _build_tiles(1)}

    # piece split: block 0 / chunks 0-3 load first so compute starts early
    QT_SPLIT = 512           # qt cols (tokens)
    KT_SPLIT = 256           # kt cols (= chunks 0-3)
    V1_SPLIT = 66 * 4        # v1 cols (= chunks 0-3)

    with tile.TileContext(nc) as tc:
        with (
            tc.tile_pool(name="qt0", bufs=2) as qt0_p,
            tc.tile_pool(name="qt1", bufs=2) as qt1_p,
            tc.tile_pool(name="kt0", bufs=2) as kt0_p,
            tc.tile_pool(name="kt1", bufs=2) as kt1_p,
            tc.tile_pool(name="v10", bufs=2) as v10_p,
            tc.tile_pool(name="v11", bufs=2) as v11_p,
            tc.tile_pool(name="ex", bufs=4) as ex_p,
            tc.tile_pool(name="otb", bufs=2) as otb_p,
            tc.tile_pool(name="sc", bufs=2, space="PSUM") as sc_p,
            tc.tile_pool(name="po", bufs=2, space="PSUM") as po_p,
        ):
            tiles_sbuf = {}

            def prep(pair, hf):
                qt0 = qt0_p.tile([128, QT_SPLIT], BF16, tag="qt0", name="qt0")
                nc.sync.dma_start(out=qt0[:, :], in_=qt_in[pair, hf][:, 0:QT_SPLIT])
                kt0 = kt0_p.tile([128, KT_SPLIT], BF16, tag="kt0", name="kt0")
                nc.sync.dma_start(out=kt0[:, :], in_=kt_in[pair, hf][:, 0:KT_SPLIT])
                v10 = v10_p.tile([128, V1_SPLIT], FP16, tag="v10", name="v10")
                nc.sync.dma_start(out=v10[:, :], in_=v1_in[pair, hf][:, 0:V1_SPLIT])
                qt1 = qt1_p.tile([128, HALF_T - QT_SPLIT], BF16, tag="qt1",
                                 name="qt1")
                nc.sync.dma_start(out=qt1[:, :], in_=qt_in[pair, hf][:, QT_SPLIT:])
                kt1 = kt1_p.tile([128, HALF_T // 2 - KT_SPLIT], BF16, tag="kt1",
                                 name="kt1")
                nc.sync.dma_start(out=kt1[:, :], in_=kt_in[pair, hf][:, KT_SPLIT:])
                v11 = v11_p.tile([128, 66 * HALF_CH - V1_SPLIT], FP16, tag="v11",
                                 name="v11")
                nc.sync.dma_start(out=v11[:, :], in_=v1_in[pair, hf][:, V1_SPLIT:])
                tiles_sbuf[(pair, hf)] = (qt0, qt1, kt0, kt1, v10, v11)

            def emit_half(pair, hf, snap0, prefetch):
                qt0, qt1, kt0, kt1, v10, v11 = tiles_sbuf.pop((pair, hf))

                def qt_ap(r0, a, b):
                    if b <= QT_SPLIT:
                        return qt0[r0:r0 + 64, a:b]
                    return qt1[r0:r0 + 64, a - QT_SPLIT:b - QT_SPLIT]

                def kt_ap(c):
                    r0, col = 64 * (c % 2), 128 * (c // 2)
                    if col < KT_SPLIT:
                        return kt0[r0:r0 + 64, col:col + 128]
                    return kt1[r0:r0 + 64, col - KT_SPLIT:col - KT_SPLIT + 128]

                def v1_ap(c):
                    col = 66 * c
                    if col < V1_SPLIT:
                        return v10[:, col:col + 65]
                    return v11[:, col - V1_SPLIT:col - V1_SPLIT + 65]
                tiles = tiles_h[hf]
                n_snap = N_SNAPS_HALF[hf]
                otb_t = otb_p.tile([65, 512 * n_snap], FP16, tag="otb",
                                   name="otb")
                state = dict(po=None, snap=0)

                def front(t):
                    sc_t = sc_p.tile([128, 1536], F32, tag="sc", name="sc")
                    t["sc"] = sc_t
                    ex_t = ex_p.tile([128, 1536], FP16, tag="ex", name="ex")
                    t["ex"] = ex_t
                    q0 = t["q0"]
                    for (c, off, nq, qo) in t["subs"]:
                        r0 = 64 * (c % 2)
                        nc.tensor.matmul(
                            sc_t[:, off:off + nq],
                            lhsT=kt_ap(c),
                            rhs=qt_ap(r0, q0 + qo, q0 + qo + nq),
                            start=True, stop=True,
                        )
                    w = t["width"]
                    if t["fe"]:
                        nc.vector.tensor_scalar(
                            out=ex_t[:, 0:w].bitcast(I16),
                            in0=sc_t[:, 0:w],
                            scalar1=float(FE_A), scalar2=float(FE_B),
                            op0=mybir.AluOpType.mult,
                            op1=mybir.AluOpType.add,
                        )
                    else:
                        nc.scalar.activation(
                            ex_t[:, 0:w], sc_t[:, 0:w],
                            mybir.ActivationFunctionType.Exp, scale=0.125,
                        )
                    for do in t["masks"]:
                        nc.gpsimd.affine_select(
                            out=ex_t[:, do:do + 128],
                            in_=ex_t[:, do:do + 128],
                            compare_op=mybir.AluOpType.is_ge,
                            fill=0.0, base=0,
                            pattern=[[1, 128]], channel_multiplier=-1,
                        )

                def back(t):
                    if t["av_first"]:
                        state["po"] = po_p.tile([65, 512], F32, tag="po",
                                                name="po")
                    po_t = state["po"]
                    ex_t = t["ex"]
                    subs = t["subs"]
                    for i, (c, off, nq, qo) in enumerate(subs):
                        nc.tensor.matmul(
                            po_t[:, qo:qo + nq],
                            lhsT=v1_ap(c),
                            rhs=ex_t[:, off:off + nq],
                            start=(t["av_first"] and i == 0),
                            stop=(t["av_last"] and i == len(subs) - 1),
                        )
                    if t["snap"]:
                        s = state["snap"]
                        dst = otb_t[0:65, 512 * s:512 * s + 512]
                        # engine balance: half0 wedge snaps on ACT (DVE busy
                        # with cross fast-exp there); everything else on DVE
                        if t["kind"] == "wedge" and hf == 0:
                            nc.scalar.copy(dst, po_t[:, :])
                        else:
                            nc.vector.tensor_copy(dst, po_t[:, :])
                        state["snap"] += 1

                backs = deque()
                for i, t in enumerate(tiles):
                    front(t)
                    if i == 2 and prefetch is not None:
                        prep(*prefetch)
                    if len(backs) >= 2:
                        back(backs.popleft())
                    backs.append(t)
                while backs:
                    back(backs.popleft())
                assert state["snap"] == n_snap
                nc.sync.dma_start(
                    out=o_out[pair][:, 512 * snap0:512 * (snap0 + n_snap)],
                    in_=otb_t[:, :],
                )

            prep(0, 0)
            for pair in range(n_pairs):
                for hf in (0, 1):
                    nxt = (pair, 1) if hf == 0 else (
                        (pair + 1, 0) if pair + 1 < n_pairs else None)
                    emit_half(pair, hf, snap0=0 if hf == 0 else N_SNAPS_HALF[0],
                              prefetch=nxt)
    _split_excess_waits(nc)
    return nc


# ------------------------------------------------------------- host wrapper
_PROGRAM = None


def _get_program():
    global _PROGRAM
    if _PROGRAM is None:
        _PROGRAM = build_program()
    return _PROGRAM


_BF = ml_dtypes.bfloat16


def _marshal(qs, ks, vs):
    """[n_pairs, 16, 512, 64] f32 triplet (slot-ordered blocks) -> device
    input dict. Pure layout/dtype marshalling - no attention math."""
    n_pairs = qs.shape[0]
    q = qs.reshape(n_pairs, 2, HALF_T, D).transpose(0, 1, 3, 2).astype(_BF)
    qt = np.ascontiguousarray(np.concatenate([q, q], axis=2))  # dup d rows

    k = ks.reshape(n_pairs, 2, HALF_CH // 2, 2, 128, D)
    kt = np.ascontiguousarray(
        k.transpose(0, 1, 3, 5, 2, 4).reshape(n_pairs, 2, 128, HALF_T // 2)
        .astype(_BF))

    v = (vs.reshape(n_pairs, 2, HALF_CH, 128, D) / 3.0).astype(np.float16)
    v1 = np.ones((n_pairs, 2, HALF_CH, 128, 66), np.float16)
    v1[..., :64] = v
    v1 = np.ascontiguousarray(
        v1.transpose(0, 1, 3, 2, 4).reshape(n_pairs, 2, 128, HALF_CH * 66))
    return {"QT": qt, "KT": kt, "V1": v1}


def _shard_inputs(Q, K, V):
    in_maps = []
    for core in range(N_CORES):
        qs, ks, vs = [], [], []
        for pi in range(PAIRS_PER_CORE):
            flat = core * PAIRS_PER_CORE + pi
            b, h = flat // NH, flat % NH
            sm = _slot_map(h)
            qs.append(Q[b, h].reshape(NBLK, BLK, D)[sm])
            ks.append(K[b, h].reshape(NBLK, BLK, D)[sm])
            vs.append(V[b, h].reshape(NBLK, BLK, D)[sm])
        in_maps.append(_marshal(np.stack(qs), np.stack(ks), np.stack(vs)))
    return in_maps


_SNAP_SLOTS = np.array(SNAP_SLOTS)


def _combine_outputs(results):
    out = np.zeros((B, NH, T, D), np.float32)
    for core in range(N_CORES):
        oc_all = results[core]["Oc"]  # [4, 23, 65, 512] fp16
        for pi in range(PAIRS_PER_CORE):
            flat = core * PAIRS_PER_CORE + pi
            b, h = flat // NH, flat % NH
            sm = _slot_map(h)
            w = _snap_weights(h % 2)
            oc = oc_all[pi].astype(np.float32).reshape(65, N_SNAPS, 512)
            num = oc[0:64].transpose(1, 0, 2)          # [23, 64, 512]
            den = oc[64][:, None, :]                   # [23, 1, 512]
            snaps = (num / den) * w[:, None, None]     # [23, 64, 512]
            slotacc = np.zeros((NBLK, BLK, D), np.float32)
            np.add.at(slotacc, _SNAP_SLOTS, snaps.transpose(0, 2, 1))
            blocks = np.empty((NBLK, BLK, D), np.float32)
            blocks[sm] = slotacc
            out[b, h] = blocks.reshape(T, D)
    return out


def kernel(Q, K, V):
    Q = np.asarray(Q, dtype=np.float32)
    K = np.asarray(K, dtype=np.float32)
    V = np.asarray(V, dtype=np.float32)
    nc = _get_program()
    in_maps = _shard_inputs(Q, K, V)
    res = run_bass_kernel_spmd(nc, in_maps, list(range(N_CORES)))
    return _combine_outputs(res.results)


if __name__ == "__main__":
    rng = np.random.default_rng(0)
    Q = rng.standard_normal((B, NH, T, D), dtype=np.float32)
    K = rng.standard_normal((B, NH, T, D), dtype=np.float32)
    V = rng.standard_normal((B, NH, T, D), dtype=np.float32)
    out = kernel(Q=Q, K=K, V=V)
    print("out", out.shape, out.dtype, float(np.abs(out).mean()))
